# revision 1
# baseline (speedup 1.0000x reference)
"""D-MPNN layer on 8 TRN2 NeuronCores (Bass/Tile, SPMD).

out = (1-z)*s + z*m with
  mess_ki = mess[nei_idx]                       [M, D]
  s_ij    = segment_sum(mess_ki, src_idx, E)    [E, D]
  z_ij    = sigmoid([h_ij | s_ij] @ Wz + bz)    [E, D]
  r_ki    = sigmoid([h_ki | mess_ki] @ Wr + br) [M, D]
  r_ij    = segment_sum(r_ki*mess_ki, src, E)   [E, D]
  m_ij    = tanh(h_ij @ W + bw + r_ij @ U)      [E, D]

Sharding: edges E split into 8 contiguous chunks (EC=E/8); each M-row is
routed on host to the core owning its src edge, so segment sums are
core-local (no collectives).  Within a core, rows (sorted by src) are
greedily packed into variable-width dst blocks: each block covers a run of
consecutive dst edges (window <= 128 wide) holding <= 384 rows (3 row
tiles), padded to exactly 384.  A final 4-tile block covers the core's last
128 dst edges.  All cores share one static program (block count padded to a
common B2 with dummy blocks); per-block dst bases live only in host-side
data (h_ij chunks, srcrel) and in the host-side un-permutation of the
block-indexed device output.

Per 128-row tile the device computes r = sigmoid(X @ Wr) (X^T streamed
pre-transposed; gathered messages pre-gathered on host), then aggregates
s^T/r^T (and row-major s) per block with one-hot matmuls
(onehot[m,d] = [srcrel[m]==d] from an iota/is_equal compare).  Per block:
z/m matmuls consume h_ij^T and the transposed s/r straight from PSUM-copied
SBUF, sigmoid/tanh, combine with row-major s, DMA out.
"""

import numpy as np
import ml_dtypes

BF16 = ml_dtypes.bfloat16

E = 262144
M = 786432
F_NB = 192
D = 256
NCORES = 8

FULL_DIMS = dict(E=E, M=M, F=F_NB, D=D, ncores=NCORES, BLK=128, C=384,
                 CT=512, KG=6)


def _dims(d, B2):
    o = dict(d)
    o["B2"] = B2
    o["EC"] = o["E"] // o["ncores"]
    o["TPB"] = o["C"] // 128              # 3 row tiles per normal block
    o["TPT"] = o["CT"] // 128             # 4 row tiles in the tail block
    assert o["KG"] == 2 * o["TPB"]
    o["BPG"] = 2
    assert B2 % 2 == 0
    o["G"] = B2 // 2
    o["T"] = o["TPB"] * B2 + o["TPT"]     # total row tiles per core
    return o


def _greedy_blocks(csum, EC, C):
    """Greedy variable-width blocks over dst edges [0, EC-128).
    csum[i] = rows with dst < i.  Returns block base list."""
    bases = []
    i = 0
    while i < EC - 128:
        base = i
        hi = min(base + 128, EC - 128)
        j = int(np.searchsorted(csum, csum[base] + C, side="right")) - 1
        j = max(base + 1, min(j, hi))
        bases.append(base)
        i = j
    return bases


def host_prep(inputs, dims=FULL_DIMS):
    dm0 = dict(dims)
    EC = dm0["E"] // dm0["ncores"]
    C, CT, KG = dm0["C"], dm0["CT"], dm0["KG"]
    F, Dd = dm0["F"], dm0["D"]
    ncores = dm0["ncores"]
    TPB = C // 128

    src = np.asarray(inputs["src_idx"]).astype(np.int64).ravel()
    nei = np.asarray(inputs["nei_idx"]).astype(np.int64).ravel()
    h_ij = np.asarray(inputs["h_ij"])
    h_ki = np.asarray(inputs["h_ki"])
    mess = np.asarray(inputs["mess"])

    order = np.argsort(src, kind="stable")
    src_s = src[order]
    nei_s = nei[order]
    cnt = np.bincount(src_s, minlength=dm0["E"])

    core_blocks = []
    for c in range(ncores):
        csum = np.concatenate(
            [[0], np.cumsum(cnt[c * EC:(c + 1) * EC])]
        )
        bases = _greedy_blocks(csum, EC, C)
        tail_rows = csum[EC] - csum[EC - 128]
        if tail_rows > CT:
            raise OverflowError(f"tail rows {tail_rows} > CT={CT}")
        core_blocks.append((bases, csum))
    nreal = [len(b[0]) for b in core_blocks]
    B2 = max(nreal)
    B2 += B2 % 2
    dm = _dims(dm0, B2)
    G, T = dm["G"], dm["T"]
    TPT = dm["TPT"]

    mess_bf = mess.astype(BF16)
    h_ki_bf = h_ki[order].astype(BF16)
    mess_g_all = mess_bf[nei_s]            # [M, D] gathered, src-sorted
    wz = np.ascontiguousarray(np.asarray(inputs["Wz_w"]).astype(BF16))
    wr = np.ascontiguousarray(np.asarray(inputs["Wr_w"]).astype(BF16))
    u = np.ascontiguousarray(np.asarray(inputs["U_w"]).astype(BF16))
    w = np.ascontiguousarray(np.asarray(inputs["W_w"]).astype(BF16))

    row_lo = np.searchsorted(src_s, np.arange(ncores) * EC)
    row_hi = np.searchsorted(src_s, (np.arange(ncores) + 1) * EC)

    in_maps = []
    metas = []
    for c in range(ncores):
        bases, csum = core_blocks[c]
        nb = len(bases)
        ndummy = B2 - nb
        MPC = B2 * C + CT
        rlo = row_lo[c]
        nrow_core = row_hi[c] - rlo

        bases_arr = np.asarray(bases, dtype=np.int64)
        nexts = np.concatenate([bases_arr[1:], [EC - 128]])
        widths = nexts - bases_arr
        rs = csum[bases_arr]               # first row of each block
        tail_start = csum[EC - 128]

        # per-row block id (for rows before the tail)
        rowblk = np.zeros(nrow_core, np.int64)
        rowblk[rs[1:][rs[1:] < nrow_core]] += 1
        rowblk = np.cumsum(rowblk)
        blk_of_row = np.minimum(rowblk, nb - 1)
        ridx = np.arange(nrow_core)
        is_tail = ridx >= tail_start
        pos_in_blk = ridx - rs[blk_of_row]
        slot_of_row = np.where(
            is_tail,
            B2 * C + (ridx - tail_start),
            (ndummy + blk_of_row) * C + pos_in_blk,
        )
        base_of_row = np.where(is_tail, EC - 128, bases_arr[blk_of_row])
        srcrel_pad = np.full(MPC, 999.0, np.float32)
        srcrel_pad[slot_of_row] = (
            src_s[rlo:row_hi[c]] - c * EC - base_of_row
        ).astype(np.float32)

        h_pad = np.zeros((MPC, F), BF16)
        h_pad[slot_of_row] = h_ki_bf[rlo:row_hi[c]]
        mg_pad = np.zeros((MPC, Dd), BF16)
        mg_pad[slot_of_row] = mess_g_all[rlo:row_hi[c]]

        # h_ij chunks per block (dummies zero), [B2+1, 128, F]
        hij_all = np.zeros((B2 + 1, 128, F), BF16)
        hijc = h_ij[c * EC:(c + 1) * EC].astype(BF16)
        gather_rows = bases_arr[:, None] + np.arange(128)[None, :]
        hij_all[ndummy:B2] = hijc[gather_rows]
        hij_all[B2] = hijc[EC - 128:]

        # ---- tile layouts ----
        NT = B2 * TPB
        src_all = np.ascontiguousarray(srcrel_pad.reshape(T, 128).T)

        def tileify(arr2d, ntiles, off_rows):
            a = arr2d[off_rows:off_rows + ntiles * 128]
            return a.reshape(ntiles, 128, -1).transpose(0, 2, 1)

        mgn = mg_pad[:NT * 128].reshape(G, KG, 128, Dd)
        mg_l = mgn.transpose(0, 2, 1, 3).reshape(G, 128, KG * Dd)
        mt3 = tileify(mg_pad, NT, 0)
        mta = (mt3[:, :128, :].reshape(G, KG, 128, 128)
               .transpose(0, 2, 1, 3).reshape(G, 128, KG * 128))
        mtb = (mt3[:, 128:, :].reshape(G, KG, 128, 128)
               .transpose(0, 2, 1, 3).reshape(G, 128, KG * 128))
        h3 = tileify(h_pad, NT, 0)
        ha = (h3[:, :128, :].reshape(G, KG, 128, 128)
              .transpose(0, 2, 1, 3).reshape(G, 128, KG * 128))
        hb = (h3[:, 128:F, :].reshape(G, KG // 2, 2, 64, 128)
              .transpose(0, 2, 3, 1, 4).reshape(G, 128, (KG // 2) * 128))
        hijt = hij_all[:B2].transpose(0, 2, 1)
        hija = (hijt[:, :128, :].reshape(G, 2, 128, 128)
                .transpose(0, 2, 1, 3).reshape(G, 128, 2 * 128))
        hijb = (hijt[:, 128:F, :].reshape(G, 1, 2, 64, 128)
                .transpose(0, 2, 3, 1, 4).reshape(G, 128, 128))
        blob = np.ascontiguousarray(
            np.concatenate([mg_l, mta, mtb, ha, hb, hija, hijb], axis=2)
        )

        # tail section (4 tiles, one block)
        toff = NT * 128
        mgt = (mg_pad[toff:].reshape(TPT, 128, Dd)
               .transpose(1, 0, 2).reshape(128, TPT * Dd))
        mtt = tileify(mg_pad, TPT, toff)
        mtta = mtt[:, :128, :].transpose(1, 0, 2).reshape(128, TPT * 128)
        mttb = mtt[:, 128:, :].transpose(1, 0, 2).reshape(128, TPT * 128)
        ht3 = tileify(h_pad, TPT, toff)
        hta = ht3[:, :128, :].transpose(1, 0, 2).reshape(128, TPT * 128)
        htb = (ht3[:, 128:F, :].reshape(2, 2, 64, 128)
               .transpose(1, 2, 0, 3).reshape(128, 2 * 128))
        htij = hij_all[B2].T
        tail = np.ascontiguousarray(
            np.concatenate([mgt, mtta, mttb, hta, htb, htij[:128]], axis=1)
        )
        htijb = np.ascontiguousarray(htij[128:F])

        in_maps.append(
            dict(srcrel=src_all, blob=blob, tail=tail, htijb=htijb,
                 wz=wz, wr=wr, u=u, w=w)
        )
        metas.append(dict(bases=bases_arr, widths=widths, ndummy=ndummy))
    return in_maps, metas, dm


def build_program(dm):
    import concourse.tile as tile
    from concourse import bacc, mybir

    EC, KG, T, G, B2 = dm["EC"], dm["KG"], dm["T"], dm["G"], dm["B2"]
    TPB, TPT, F, Dd = dm["TPB"], dm["TPT"], dm["F"], dm["D"]
    f32 = mybir.dt.float32
    bf16 = mybir.dt.bfloat16
    i32 = mybir.dt.int32
    AF = mybir.ActivationFunctionType
    ALU = mybir.AluOpType

    nc = bacc.Bacc("TRN2", target_bir_lowering=False, debug=False,
                   num_devices=dm["ncores"])

    SEG = [KG * Dd, KG * 128, KG * 128, KG * 128, (KG // 2) * 128,
           2 * 128, 128]
    SEGOFF = [0]
    for sgl in SEG:
        SEGOFF.append(SEGOFF[-1] + sgl)
    SEGT = [TPT * Dd, TPT * 128, TPT * 128, TPT * 128, 2 * 128, 128]
    SEGTOFF = [0]
    for sgl in SEGT:
        SEGTOFF.append(SEGTOFF[-1] + sgl)

    srcrel_d = nc.dram_tensor("srcrel", [128, T], f32, kind="ExternalInput")
    blob_d = nc.dram_tensor("blob", [G, 128, SEGOFF[-1]], bf16,
                            kind="ExternalInput")
    tail_d = nc.dram_tensor("tail", [128, SEGTOFF[-1]], bf16,
                            kind="ExternalInput")
    htijb_d = nc.dram_tensor("htijb", [64, 128], bf16, kind="ExternalInput")
    wz_d = nc.dram_tensor("wz", [F + Dd, Dd], bf16, kind="ExternalInput")
    wr_d = nc.dram_tensor("wr", [F + Dd, Dd], bf16, kind="ExternalInput")
    u_d = nc.dram_tensor("u", [Dd, Dd], bf16, kind="ExternalInput")
    w_d = nc.dram_tensor("w", [F, Dd], bf16, kind="ExternalInput")
    y_d = nc.dram_tensor("y", [(B2 + 1) * 128, Dd], f32,
                         kind="ExternalOutput")

    with tile.TileContext(nc) as tc:
        with (
            tc.tile_pool(name="const", bufs=1) as const,
            tc.tile_pool(name="gat", bufs=4) as gat,
            tc.tile_pool(name="mid", bufs=4) as mid,
            tc.tile_pool(name="fin", bufs=4) as fin,
            tc.tile_pool(name="psA", bufs=2, space="PSUM") as psA,
            tc.tile_pool(name="psS", bufs=4, space="PSUM") as psS,
            tc.tile_pool(name="psR", bufs=2, space="PSUM") as psR,
        ):
            iota_i = const.tile([128, 128], i32)
            nc.gpsimd.iota(iota_i[:], pattern=[[1, 128]], base=0,
                           channel_multiplier=0)
            iota_f = const.tile([128, 128], f32)
            nc.vector.tensor_copy(iota_f[:], iota_i[:])

            def load_w(dram, ks, nm):
                tiles = []
                r0 = 0
                for i, k in enumerate(ks):
                    t = const.tile([k, Dd], bf16, tag=f"{nm}{i}")
                    nc.sync.dma_start(out=t[:], in_=dram[r0:r0 + k, :])
                    tiles.append(t)
                    r0 += k
                return tiles

            wr_t = load_w(wr_d, (128, 64, 128, 128), "wr")
            wr1d = const.tile([128, Dd], bf16, tag="wr1d")
            nc.sync.dma_start(out=wr1d[0:64, :], in_=wr_d[128:192, :])
            nc.sync.dma_start(out=wr1d[64:128, :], in_=wr_d[128:192, :])
            wz_t = load_w(wz_d, (128, 64, 128, 128), "wz")
            w_t = load_w(w_d, (128, 64), "w")
            u_t = load_w(u_d, (128, 128), "u")
            zw0 = const.tile([128, 2 * Dd], bf16, tag="zw0")
            nc.sync.dma_start(out=zw0[:, 0:Dd], in_=wz_d[0:128, :])
            nc.sync.dma_start(out=zw0[:, Dd:2 * Dd], in_=w_d[0:128, :])
            zw1 = const.tile([128, 2 * Dd], bf16, tag="zw1")
            for half in (0, 64):
                nc.sync.dma_start(out=zw1[half:half + 64, 0:Dd],
                                  in_=wz_d[128:192, :])
                nc.sync.dma_start(out=zw1[half:half + 64, Dd:2 * Dd],
                                  in_=w_d[128:192, :])

            src_all = const.tile([128, T], f32)
            nc.sync.dma_start(out=src_all[:], in_=srcrel_d[:, :])

            def do_tiles(ntile, t0, mess_g, mta_sb, mtb_sb, ha_sb, hb_sb):
                oh_g = mid.tile([128, KG, 128], bf16, tag="oh")
                nc.vector.tensor_tensor(
                    out=oh_g[:, :ntile, :],
                    in0=src_all[:, t0:t0 + ntile, None].broadcast_to(
                        [128, ntile, 128]),
                    in1=iota_f[:, None, :].broadcast_to([128, ntile, 128]),
                    op=ALU.is_equal,
                )
                r_g = mid.tile([128, KG * Dd], bf16, tag="rg")
                for j in range(ntile):
                    pr = psS.tile([128, Dd], f32, tag="ps")
                    nc.tensor.matmul(out=pr[:],
                                     lhsT=ha_sb[:, j * 128:(j + 1) * 128],
                                     rhs=wr_t[0][:], start=True, stop=False)
                    half = (j % 2) * 64
                    hb_t = hb_sb[half:half + 64,
                                 (j // 2) * 128:(j // 2 + 1) * 128]
                    nc.tensor.matmul(out=pr[:], lhsT=hb_t,
                                     rhs=wr1d[half:half + 64, :],
                                     start=False, stop=False)
                    nc.tensor.matmul(out=pr[:],
                                     lhsT=mta_sb[:, j * 128:(j + 1) * 128],
                                     rhs=wr_t[2][:], start=False, stop=False)
                    nc.tensor.matmul(out=pr[:],
                                     lhsT=mtb_sb[:, j * 128:(j + 1) * 128],
                                     rhs=wr_t[3][:], start=False, stop=True)
                    nc.scalar.activation(r_g[:, j * Dd:(j + 1) * Dd], pr[:],
                                         AF.Sigmoid)
                rm_g = mid.tile([128, KG * Dd], bf16, tag="rm")
                nc.vector.tensor_tensor(out=rm_g[:, :ntile * Dd],
                                        in0=r_g[:, :ntile * Dd],
                                        in1=mess_g[:, :ntile * Dd],
                                        op=ALU.mult)
                return r_g, rm_g, oh_g

            def do_block(b, ntile, j0, oh_g, mess_g, rm_g,
                         hija_t, hijb_t, bhalf):
                pa = psA.tile([128, 4 * 128], f32, tag="pa")
                psr = psR.tile([128, Dd], f32, tag="psr")
                for tj in range(ntile):
                    j = j0 + tj
                    oh = oh_g[:, j, :]
                    mess_t = mess_g[:, j * Dd:(j + 1) * Dd]
                    rm_t = rm_g[:, j * Dd:(j + 1) * Dd]
                    st = tj == 0
                    sp = tj == ntile - 1
                    nc.tensor.matmul(out=pa[:, 0:128], lhsT=mess_t[:, 0:128],
                                     rhs=oh, start=st, stop=False)
                    nc.tensor.matmul(out=pa[:, 128:256],
                                     lhsT=mess_t[:, 128:256],
                                     rhs=oh, start=False, stop=False)
                    nc.tensor.matmul(out=pa[:, 256:384], lhsT=rm_t[:, 0:128],
                                     rhs=oh, start=False, stop=False)
                    nc.tensor.matmul(out=pa[:, 384:512],
                                     lhsT=rm_t[:, 128:256],
                                     rhs=oh, start=False, stop=sp)
                    nc.tensor.matmul(out=psr[:], lhsT=oh, rhs=mess_t,
                                     start=st, stop=sp)

                sr_sb = fin.tile([128, 512], bf16, tag="sr")
                nc.vector.tensor_copy(sr_sb[:, 0:256], pa[:, 0:256])
                nc.scalar.copy(sr_sb[:, 256:512], pa[:, 256:512])

                pzm = psS.tile([128, 512], f32, tag="ps")
                nc.tensor.matmul(out=pzm[:, 0:512], lhsT=hija_t,
                                 rhs=zw0[:], start=True, stop=False)
                nc.tensor.matmul(out=pzm[:, 0:512], lhsT=hijb_t,
                                 rhs=zw1[bhalf:bhalf + 64, :],
                                 start=False, stop=False)
                nc.tensor.matmul(out=pzm[:, 0:256], lhsT=sr_sb[:, 0:128],
                                 rhs=wz_t[2][:], start=False, stop=False)
                nc.tensor.matmul(out=pzm[:, 0:256], lhsT=sr_sb[:, 128:256],
                                 rhs=wz_t[3][:], start=False, stop=False)
                nc.tensor.matmul(out=pzm[:, 256:512], lhsT=sr_sb[:, 256:384],
                                 rhs=u_t[0][:], start=False, stop=False)
                nc.tensor.matmul(out=pzm[:, 256:512], lhsT=sr_sb[:, 384:512],
                                 rhs=u_t[1][:], start=False, stop=True)

                z_sb = fin.tile([128, Dd], f32, tag="z")
                nc.scalar.activation(z_sb[:], pzm[:, 0:256], AF.Sigmoid)
                m_sb = fin.tile([128, Dd], f32, tag="m")
                nc.scalar.activation(m_sb[:], pzm[:, 256:512], AF.Tanh)

                d_sb = fin.tile([128, Dd], f32, tag="d")
                nc.vector.tensor_tensor(out=d_sb[:], in0=m_sb[:], in1=psr[:],
                                        op=ALU.subtract)
                nc.vector.tensor_tensor(out=d_sb[:], in0=d_sb[:], in1=z_sb[:],
                                        op=ALU.mult)
                o_sb = fin.tile([128, Dd], f32, tag="o")
                nc.vector.tensor_tensor(out=o_sb[:], in0=d_sb[:], in1=psr[:],
                                        op=ALU.add)
                nc.scalar.dma_start(out=y_d[b * 128:(b + 1) * 128, :],
                                    in_=o_sb[:])

            for g in range(G):
                blob_sb = gat.tile([128, SEGOFF[-1]], bf16, tag="blob")
                nc.sync.dma_start(out=blob_sb[:], in_=blob_d[g])
                mess_g = blob_sb[:, SEGOFF[0]:SEGOFF[1]]
                mta_sb = blob_sb[:, SEGOFF[1]:SEGOFF[2]]
                mtb_sb = blob_sb[:, SEGOFF[2]:SEGOFF[3]]
                ha_sb = blob_sb[:, SEGOFF[3]:SEGOFF[4]]
                hb_sb = blob_sb[:, SEGOFF[4]:SEGOFF[5]]
                hija_sb = blob_sb[:, SEGOFF[5]:SEGOFF[6]]
                hijb_sb = blob_sb[:, SEGOFF[6]:SEGOFF[7]]

                r_g, rm_g, oh_g = do_tiles(KG, g * KG, mess_g, mta_sb,
                                           mtb_sb, ha_sb, hb_sb)
                for bb in range(2):
                    do_block(
                        g * 2 + bb, TPB, bb * TPB, oh_g, mess_g, rm_g,
                        hija_sb[:, bb * 128:(bb + 1) * 128],
                        hijb_sb[(bb % 2) * 64:(bb % 2) * 64 + 64, 0:128],
                        (bb % 2) * 64,
                    )

            # tail block
            tail_sb = gat.tile([128, SEGTOFF[-1]], bf16, tag="tail")
            nc.sync.dma_start(out=tail_sb[:], in_=tail_d[:, :])
            htijb_sb = gat.tile([64, 128], bf16, tag="htijb")
            nc.sync.dma_start(out=htijb_sb[:], in_=htijb_d[:, :])
            r_g, rm_g, oh_g = do_tiles(
                TPT, B2 * TPB,
                tail_sb[:, SEGTOFF[0]:SEGTOFF[1]],
                tail_sb[:, SEGTOFF[1]:SEGTOFF[2]],
                tail_sb[:, SEGTOFF[2]:SEGTOFF[3]],
                tail_sb[:, SEGTOFF[3]:SEGTOFF[4]],
                tail_sb[:, SEGTOFF[4]:SEGTOFF[5]],
            )
            do_block(B2, TPT, 0, oh_g,
                     tail_sb[:, SEGTOFF[0]:SEGTOFF[1]], rm_g,
                     tail_sb[:, SEGTOFF[5]:SEGTOFF[6]],
                     htijb_sb[0:64, 0:128], 0)

    nc.compile()
    return nc


_CACHE = {}
LAST_RESULT = None


def kernel(**inputs):
    from concourse.bass_utils import run_bass_kernel_spmd

    for b in ("Wz_b", "Wr_b", "W_b"):
        assert not np.any(np.asarray(inputs[b])), f"nonzero bias {b} unsupported"

    in_maps, metas, dm = host_prep(inputs, FULL_DIMS)
    key = (tuple(sorted(FULL_DIMS.items())), dm["B2"])
    if key not in _CACHE:
        _CACHE[key] = build_program(dm)
    nc = _CACHE[key]
    import os
    trace = os.environ.get("DMPNN_TRACE", "") == "1"
    res = run_bass_kernel_spmd(nc, in_maps, core_ids=list(range(dm["ncores"])),
                               trace=trace, trace_cores=[0] if trace else None)
    global LAST_RESULT
    LAST_RESULT = res

    EC = dm["EC"]
    B2 = dm["B2"]
    out = np.empty((dm["E"], dm["D"]), np.float32)
    for c in range(dm["ncores"]):
        yc = res.results[c]["y"]
        meta = metas[c]
        oc = out[c * EC:(c + 1) * EC]
        nd = meta["ndummy"]
        for i, (base, wdt) in enumerate(zip(meta["bases"], meta["widths"])):
            b = nd + i
            oc[base:base + wdt] = yc[b * 128:b * 128 + wdt]
        oc[EC - 128:] = yc[B2 * 128:(B2 + 1) * 128]
    return out

